# revision 12
# baseline (speedup 1.0000x reference)
"""Trainium2 Bass kernel for ConvPosDivMultiHeadAttn (B=8, L=512, D=1024, H=16).

Sharding: pure data-parallel over batch — 8 cores, 1 batch element each, all
16 heads on-core, weights replicated. No collectives.

Dtypes: fp16 operands for all projection/score matmuls (full PE rate, 11-bit
mantissa), fp32 PSUM accumulation, bf16 for exp outputs / V (needs dynamic
range up to e^~35), fp32 final output.

Per-core pipeline:
  1. x (fp16) -> SBUF, transpose to xT [d, t] via PE transposes.
  2. Feature-major q/k projection qkT[f, t] = w_qk^T-as-lhsT @ xT, with the
     speaker mask fused into the PSUM->SBUF copy as a rank-2 decomposition:
       qsame[i,j] = u_i u_j + (1-u_i)(1-u_j)   (u = qmask in {0,1})
     so (S+pos) * qsame needs masked copies Q1/Q0/K1/K0 (and likewise the
     positional projections QP1/QP0/KP1/KP0).
  3. v projected token-major into a ones-augmented V (extra column of 1s per
     head) so the AV matmul also yields the softmax denominator.
  4. Scores computed TRANSPOSED, S^T[j, i]: 5 accumulating matmuls per
     (head, j-tile): k1q1 + k0q0 + kp1qp1 + kp0qp0 + I @ A'^T, where A'^T is
     host-precomputed: gaussian bias + key-padding(-inf in fp16) - per-row
     stabilizer c_base[i] (rowmax of gaussian+padding, within ~±30 of the
     true row max - safe for fp32 exp range).
  5. E^T = exp(S'^T) on the scalar engine (PSUM -> bf16 SBUF).
  6. out^T[d, i] (+ sums row) = V_aug-as-lhsT @ E^T; normalization via vector
     reciprocal of the sums row, K=1 matmul broadcast, multiply fused into
     the PSUM->SBUF copy building out_attn^T feature-major (fp16).
  7. y = out_attn^T-as-lhsT @ w_fc + b_fc (fp32 out, bias fused).
"""

import sys

import numpy as np

sys.path.insert(0, "/opt/trn_rl_repo")

import concourse.bass as bass  # noqa: E402
import concourse.tile as tile  # noqa: E402
from concourse import bacc, mybir  # noqa: E402
from concourse.masks import make_identity  # noqa: E402

B, L, D, H = 8, 512, 1024, 16
HD = D // H  # 64
FP = mybir.dt.float32
F16 = mybir.dt.float16
BF = mybir.dt.bfloat16


def build_kernel(nc):
    """Emit the single-core program. All loops static/unrolled under Tile."""
    from contextlib import ExitStack

    AF = mybir.ActivationFunctionType
    OP = mybir.AluOpType

    x = nc.dram_tensor("x", [L, D], F16, kind="ExternalInput").ap()
    wqkv = nc.dram_tensor("wqkv", [D, 3 * D], F16, kind="ExternalInput").ap()
    wqp = nc.dram_tensor("wqp", [HD, 2 * D], F16, kind="ExternalInput").ap()
    wfc = nc.dram_tensor("wfc", [D, D], F16, kind="ExternalInput").ap()
    peT = nc.dram_tensor("peT", [HD, L], F16, kind="ExternalInput").ap()
    at = nc.dram_tensor("at", [L, L], F16, kind="ExternalInput").ap()
    U1 = nc.dram_tensor("U1", [128, L], F16, kind="ExternalInput").ap()
    U0 = nc.dram_tensor("U0", [128, L], F16, kind="ExternalInput").ap()
    BB = nc.dram_tensor("BB", [128, D], FP, kind="ExternalInput").ap()
    y = nc.dram_tensor("y", [L, D], FP, kind="ExternalOutput").ap()

    with tile.TileContext(nc) as tc:
        with ExitStack() as ctx:
            ctx.enter_context(
                nc.allow_low_precision(reason="fp16/bf16 operand pipeline by design")
            )
            const = ctx.enter_context(tc.tile_pool(name="const", bufs=1))
            xpool = ctx.enter_context(tc.tile_pool(name="xp", bufs=1))
            big = ctx.enter_context(tc.tile_pool(name="big", bufs=1))
            wqs = ctx.enter_context(tc.tile_pool(name="wqs", bufs=8))
            wvs = ctx.enter_context(tc.tile_pool(name="wvs", bufs=8))
            atp = ctx.enter_context(tc.tile_pool(name="atp", bufs=4))
            etp = ctx.enter_context(tc.tile_pool(name="etp", bufs=8))
            ysb = ctx.enter_context(tc.tile_pool(name="ysb", bufs=4))
            rcp = ctx.enter_context(tc.tile_pool(name="rcp", bufs=4))
            pp = ctx.enter_context(tc.tile_pool(name="pp", bufs=2, space="PSUM"))
            sp = ctx.enter_context(tc.tile_pool(name="sp", bufs=2, space="PSUM"))
            ap_ = ctx.enter_context(tc.tile_pool(name="ap", bufs=2, space="PSUM"))
            rp = ctx.enter_context(tc.tile_pool(name="rp", bufs=2, space="PSUM"))

            # ---- constants ----
            ident = const.tile([128, 128], F16)
            make_identity(nc, ident[:])
            ones64 = const.tile([128, 64], F16)
            nc.vector.memset(ones64[:], 1.0)
            u1t = const.tile([128, L], F16)
            nc.sync.dma_start(u1t[:], U1)
            u0t = const.tile([128, L], F16)
            nc.sync.dma_start(u0t[:], U0)
            bbt = const.tile([128, D], FP)
            nc.sync.dma_start(bbt[:], BB)
            pet = const.tile([HD, L], F16)
            nc.sync.dma_start(pet[:], peT)
            wqpt = const.tile([HD, 2 * D], F16)
            nc.sync.dma_start(wqpt[:], wqp)

            # ---- phase 1: load x, build xT [d, t] (free: dc*512 + tc*128) ----
            xtok = xpool.tile([128, 4 * D], F16)  # [:, tc*1024 : +1024]
            for tc_ in range(4):
                nc.sync.dma_start(
                    xtok[:, tc_ * D : (tc_ + 1) * D], x[tc_ * 128 : (tc_ + 1) * 128, :]
                )
            xT = xpool.tile([128, 8 * 512], F16)
            for dc in range(8):
                pb = pp.tile([128, 512], F16, tag="pp")
                for tc_ in range(4):
                    nc.tensor.transpose(
                        pb[:, tc_ * 128 : (tc_ + 1) * 128],
                        xtok[:, tc_ * D + dc * 128 : tc_ * D + (dc + 1) * 128],
                        ident[:],
                    )
                nc.vector.tensor_copy(xT[:, dc * 512 : (dc + 1) * 512], pb[:])

            # ---- phase 3: v projection, token-major, ones-augmented (bf16) ----
            # vaug: [128, 4*16*65]; (tc, h) block at (tc*16+h)*65, col 64 = 1.0
            vaug = big.tile([128, 4 * 16 * 65], BF)
            v3 = vaug[:].rearrange("p (c e) -> p c e", e=65)
            nc.vector.memset(v3[:, :, 64:65], 1.0)
            for nv in range(2):
                wvt = []
                for kc in range(8):
                    wv = wvs.tile([128, 512], F16, tag="wv")
                    nc.sync.dma_start(
                        wv[:],
                        wqkv[
                            kc * 128 : (kc + 1) * 128,
                            2 * D + nv * 512 : 2 * D + (nv + 1) * 512,
                        ],
                    )
                    wvt.append(wv)
                for tc_ in range(4):
                    vp = pp.tile([128, 512], FP, tag="pp")
                    for kc in range(8):
                        nc.tensor.matmul(
                            vp[:],
                            xT[:, kc * 512 + tc_ * 128 : kc * 512 + tc_ * 128 + 128],
                            wvt[kc][:],
                            start=(kc == 0),
                            stop=(kc == 7),
                        )
                    nc.vector.tensor_copy(
                        v3[:, tc_ * 16 + nv * 8 : tc_ * 16 + (nv + 1) * 8, 0:64], vp[:]
                    )

            # ---- A'^T tiles (gauss + key padding - row stabilizer), loaded once ----
            at_sb = []
            for jt in range(4):
                a = atp.tile([128, 512], F16)
                nc.sync.dma_start(a[:], at[jt * 128 : (jt + 1) * 128, :])
                at_sb.append(a)

            oaT = big.tile([128, 8 * 512], F16)  # out_attn^T, feature-major

            # ---- phases 2,4,5-7 per head-group of 8 ----
            # Within a group: local head hl at rows (hl%2)*64..+64,
            # cols (hl//2)*512..+512 of Q1/Q0/K1/K0/QP1/QP0/KP1/KP0.
            for g in range(2):
                Q1 = big.tile([128, 4 * 512], F16, tag="Q1")
                Q0 = big.tile([128, 4 * 512], F16, tag="Q0")
                K1 = big.tile([128, 4 * 512], F16, tag="K1")
                K0 = big.tile([128, 4 * 512], F16, tag="K0")
                QP1 = big.tile([128, 4 * 512], F16, tag="QP1")
                QP0 = big.tile([128, 4 * 512], F16, tag="QP0")
                KP1 = big.tile([128, 4 * 512], F16, tag="KP1")
                KP0 = big.tile([128, 4 * 512], F16, tag="KP0")

                # q/k projection (feature-major) + speaker-mask fused copies
                for fpt in range(8):
                    # local feature-tiles 0-3 = q of heads g*8.., 4-7 = k
                    isq = fpt < 4
                    p = fpt if isq else fpt - 4
                    col = (0 if isq else D) + g * 512 + p * 128
                    qp_ps = pp.tile([128, 512], FP, tag="pp")
                    for kc in range(8):
                        wt = wqs.tile([128, 128], F16)
                        nc.sync.dma_start(
                            wt[:], wqkv[kc * 128 : (kc + 1) * 128, col : col + 128]
                        )
                        nc.tensor.matmul(
                            qp_ps[:],
                            wt[:],
                            xT[:, kc * 512 : (kc + 1) * 512],
                            start=(kc == 0),
                            stop=(kc == 7),
                        )
                    d1, d0 = (Q1, Q0) if isq else (K1, K0)
                    sl = slice(p * 512, (p + 1) * 512)
                    nc.vector.tensor_mul(d1[:, sl], qp_ps[:], u1t[:])
                    nc.vector.tensor_mul(d0[:, sl], qp_ps[:], u0t[:])

                # positional projection (feature-major) + mask fused copies
                for fpt in range(8):
                    isq = fpt < 4
                    p = fpt if isq else fpt - 4
                    col = (0 if isq else D) + g * 512 + p * 128
                    yp = pp.tile([128, 512], FP, tag="pp")
                    nc.tensor.matmul(
                        yp[:],
                        wqpt[:, col : col + 128],
                        pet[:],
                        start=True,
                        stop=True,
                    )
                    d1, d0 = (QP1, QP0) if isq else (KP1, KP0)
                    sl = slice(p * 512, (p + 1) * 512)
                    nc.vector.tensor_mul(d1[:, sl], yp[:], u1t[:])
                    nc.vector.tensor_mul(d0[:, sl], yp[:], u0t[:])

                # scores^T -> exp -> AV -> normalize, per local head
                for hl in range(8):
                    h = g * 8 + hl
                    hb = (hl % 2) * 64
                    hp = (hl // 2) * 512
                    ets = []
                    for jt in range(4):
                        s_ps = sp.tile([128, 512], FP, tag="sp")
                        jsl = slice(hp + jt * 128, hp + jt * 128 + 128)
                        isl = slice(hp, hp + 512)
                        nc.tensor.matmul(
                            s_ps[:], K1[hb : hb + 64, jsl], Q1[hb : hb + 64, isl],
                            start=True, stop=False,
                        )
                        nc.tensor.matmul(
                            s_ps[:], K0[hb : hb + 64, jsl], Q0[hb : hb + 64, isl],
                            start=False, stop=False,
                        )
                        nc.tensor.matmul(
                            s_ps[:], KP1[hb : hb + 64, jsl], QP1[hb : hb + 64, isl],
                            start=False, stop=False,
                        )
                        nc.tensor.matmul(
                            s_ps[:], KP0[hb : hb + 64, jsl], QP0[hb : hb + 64, isl],
                            start=False, stop=False,
                        )
                        nc.tensor.matmul(
                            s_ps[:], ident[:], at_sb[jt][:],
                            start=False, stop=True,
                        )
                        e_t = etp.tile([128, 512], BF)
                        nc.scalar.activation(e_t[:], s_ps[:], AF.Exp)
                        ets.append(e_t)

                    av = ap_.tile([128, 512], FP, tag="ap")
                    for jt in range(4):
                        base = jt * 16 * 65 + h * 65
                        nc.tensor.matmul(
                            av[0:65, :],
                            vaug[:, base : base + 65],
                            ets[jt][:],
                            start=(jt == 0),
                            stop=(jt == 3),
                        )
                    rec = rcp.tile([128, 512], F16)
                    nc.vector.reciprocal(rec[64:65, :], av[64:65, :])
                    rb = rp.tile([64, 512], FP, tag="rp")
                    nc.tensor.matmul(
                        rb[:], ones64[64:65, 0:64], rec[64:65, :],
                        start=True, stop=True,
                    )
                    rbs = rcp.tile([64, 512], FP, tag="rbs")
                    nc.scalar.copy(rbs[:], rb[:])
                    ob = (h % 2) * 64
                    op = (h // 2) * 512
                    nc.vector.tensor_mul(
                        oaT[ob : ob + 64, op : op + 512], av[0:64, :], rbs[:]
                    )

            # ---- phase 8: FC + bias ----
            for ne in range(2):
                wft = []
                for fc8 in range(8):
                    wf = wvs.tile([128, 512], F16, tag="wv")
                    nc.sync.dma_start(
                        wf[:],
                        wfc[fc8 * 128 : (fc8 + 1) * 128, ne * 512 : (ne + 1) * 512],
                    )
                    wft.append(wf)
                for tc_ in range(4):
                    yp_ = ap_.tile([128, 512], FP, tag="ap")
                    for fc8 in range(8):
                        nc.tensor.matmul(
                            yp_[:],
                            oaT[:, fc8 * 512 + tc_ * 128 : fc8 * 512 + tc_ * 128 + 128],
                            wft[fc8][:],
                            start=(fc8 == 0),
                            stop=(fc8 == 7),
                        )
                    y_t = ysb.tile([128, 512], FP)
                    nc.vector.scalar_tensor_tensor(
                        y_t[:], yp_[:], 1.0, bbt[:, ne * 512 : (ne + 1) * 512],
                        op0=OP.mult, op1=OP.add,
                    )
                    nc.sync.dma_start(
                        y[tc_ * 128 : (tc_ + 1) * 128, ne * 512 : (ne + 1) * 512],
                        y_t[:],
                    )
    return nc


def host_prep(x, mask, qmask, w_qkv, w_qkpos, w_fc, b_fc, shift, bias):
    """Build per-core input maps (host-side numpy only)."""
    x = np.asarray(x, np.float32)
    mask = np.asarray(mask)
    qmask = np.asarray(qmask)
    b_fc = np.asarray(b_fc, np.float32)
    shift = float(np.asarray(shift).reshape(-1)[0])
    bias = float(np.asarray(bias).reshape(-1)[0])
    wqkv16 = np.asarray(w_qkv).astype(np.float16)
    wqp16 = np.asarray(w_qkpos).astype(np.float16)
    wfc16 = np.asarray(w_fc).astype(np.float16)

    half = HD // 2
    inv = np.exp(np.arange(half, dtype=np.float64) * (-(np.log(10000.0) / (half - 1))))
    r = np.arange(-(L // 2), L // 2, dtype=np.float64)
    ang = r[:, None] * inv[None, :]
    pe = np.concatenate([np.sin(ang), np.cos(ang)], axis=1).astype(np.float32)
    peT16 = np.ascontiguousarray(pe.T).astype(np.float16)  # (HD, L)

    idx = np.arange(L, dtype=np.float32)
    sqd = (idx[:, None] - idx[None, :]) ** 2
    G = -(shift * sqd + bias)  # (L, L), symmetric

    BBrow = np.ascontiguousarray(
        np.broadcast_to(b_fc[None, :], (128, D)).astype(np.float32)
    )

    in_maps = []
    for b in range(B):
        kneg = np.where(mask[b] == 0, np.float32(-1.0e9), np.float32(0.0))
        c_base = (G + kneg[None, :]).max(axis=1)  # max over valid j
        aT = (G + kneg[:, None] - c_base[None, :]).astype(np.float32)  # [j, i]
        # clamp to finite fp16 (avoid 0*inf=NaN in the identity matmul);
        # -60000 + |S|max is still << ln(fp32 min), so exp -> 0 exactly
        aT16 = np.clip(aT, -60000.0, None).astype(np.float16)
        u = qmask[b].astype(np.float16)
        U1 = np.ascontiguousarray(np.broadcast_to(u[None, :], (128, L)))
        U0 = np.ascontiguousarray(np.broadcast_to((1 - u)[None, :], (128, L)))
        in_maps.append(
            dict(
                x=np.ascontiguousarray(x[b]).astype(np.float16),
                wqkv=wqkv16,
                wqp=wqp16,
                wfc=wfc16,
                peT=peT16,
                at=np.ascontiguousarray(aT16),
                U1=U1,
                U0=U0,
                BB=BBrow,
            )
        )
    return in_maps


_NC_CACHE = {}


def get_nc():
    if "nc" not in _NC_CACHE:
        nc = bacc.Bacc(
            "TRN2", target_bir_lowering=False, debug=False, enable_asserts=False,
            num_devices=B,
        )
        build_kernel(nc)
        nc.compile()
        _NC_CACHE["nc"] = nc
    return _NC_CACHE["nc"]


def kernel(**inputs):
    from concourse import bass_utils

    in_maps = host_prep(**inputs)
    nc = get_nc()
    res = bass_utils.run_bass_kernel_spmd(nc, in_maps, list(range(B)))
    out = np.stack([m["y"] for m in res.results], axis=0)
    return out.astype(np.float32)


if __name__ == "__main__":
    rng = np.random.default_rng(0)
    ins = dict(
        x=rng.standard_normal((B, L, D), dtype=np.float32),
        mask=rng.integers(0, 2, (B, L)).astype(np.int64),
        qmask=rng.integers(0, 2, (B, L)).astype(np.int64),
        w_qkv=(rng.standard_normal((D, 3 * D), dtype=np.float32) * 0.02),
        w_qkpos=(rng.standard_normal((HD, 2 * D), dtype=np.float32) * 0.02),
        w_fc=(rng.standard_normal((D, D), dtype=np.float32) * 0.02),
        b_fc=np.zeros((D,), np.float32),
        shift=np.abs(rng.standard_normal(1)).astype(np.float32) + 0.001,
        bias=-np.abs(rng.standard_normal(1)).astype(np.float32),
    )
    ins["mask"][:, 0] = 1
    out = kernel(**ins)
    print(out.shape, out.dtype)


# revision 13
# speedup vs baseline: 329.2769x; 329.2769x over previous
"""Trainium2 Bass kernel for ConvPosDivMultiHeadAttn (B=8, L=512, D=1024, H=16).

Sharding: pure data-parallel over batch — 8 cores, 1 batch element each, all
16 heads on-core, weights replicated. No collectives.

Dtypes: fp16 operands for all projection/score matmuls (full PE rate, 11-bit
mantissa), fp32 PSUM accumulation, bf16 for exp outputs / V (needs dynamic
range up to e^~35), fp32 final output.

Per-core pipeline:
  1. x (fp16) -> SBUF, transpose to xT [d, t] via PE transposes.
  2. Feature-major q/k projection qkT[f, t] = w_qk^T-as-lhsT @ xT, with the
     speaker mask fused into the PSUM->SBUF copy as a rank-2 decomposition:
       qsame[i,j] = u_i u_j + (1-u_i)(1-u_j)   (u = qmask in {0,1})
     so (S+pos) * qsame needs masked copies Q1/Q0/K1/K0 (and likewise the
     positional projections QP1/QP0/KP1/KP0).
  3. v projected token-major into a ones-augmented V (extra column of 1s per
     head) so the AV matmul also yields the softmax denominator.
  4. Scores computed TRANSPOSED, S^T[j, i]: 5 accumulating matmuls per
     (head, j-tile): k1q1 + k0q0 + kp1qp1 + kp0qp0 + I @ A'^T, where A'^T is
     host-precomputed: gaussian bias + key-padding(-inf in fp16) - per-row
     stabilizer c_base[i] (rowmax of gaussian+padding, within ~±30 of the
     true row max - safe for fp32 exp range).
  5. E^T = exp(S'^T) on the scalar engine (PSUM -> bf16 SBUF).
  6. out^T[d, i] (+ sums row) = V_aug-as-lhsT @ E^T; normalization via vector
     reciprocal of the sums row, K=1 matmul broadcast, multiply fused into
     the PSUM->SBUF copy building out_attn^T feature-major (fp16).
  7. y = out_attn^T-as-lhsT @ w_fc + b_fc (fp32 out, bias fused).
"""

import sys

import numpy as np

sys.path.insert(0, "/opt/trn_rl_repo")

import concourse.bass as bass  # noqa: E402
import concourse.tile as tile  # noqa: E402
from concourse import bacc, mybir  # noqa: E402
from concourse.masks import make_identity  # noqa: E402

B, L, D, H = 8, 512, 1024, 16
HD = D // H  # 64
FP = mybir.dt.float32
F16 = mybir.dt.float16
BF = mybir.dt.bfloat16


def build_kernel(nc):
    """Emit the single-core program. All loops static/unrolled under Tile."""
    from contextlib import ExitStack

    AF = mybir.ActivationFunctionType
    OP = mybir.AluOpType

    x = nc.dram_tensor("x", [L, D], F16, kind="ExternalInput").ap()
    wqkv = nc.dram_tensor("wqkv", [D, 3 * D], F16, kind="ExternalInput").ap()
    wqp = nc.dram_tensor("wqp", [HD, 2 * D], F16, kind="ExternalInput").ap()
    wfc = nc.dram_tensor("wfc", [D, D], F16, kind="ExternalInput").ap()
    peT = nc.dram_tensor("peT", [HD, L], F16, kind="ExternalInput").ap()
    at = nc.dram_tensor("at", [L, L], F16, kind="ExternalInput").ap()
    U1 = nc.dram_tensor("U1", [128, L], F16, kind="ExternalInput").ap()
    U0 = nc.dram_tensor("U0", [128, L], F16, kind="ExternalInput").ap()
    BB = nc.dram_tensor("BB", [128, D], FP, kind="ExternalInput").ap()
    y = nc.dram_tensor("y", [L, D], FP, kind="ExternalOutput").ap()

    with tile.TileContext(nc) as tc:
        with ExitStack() as ctx:
            ctx.enter_context(
                nc.allow_low_precision(reason="fp16/bf16 operand pipeline by design")
            )
            const = ctx.enter_context(tc.tile_pool(name="const", bufs=1))
            xpool = ctx.enter_context(tc.tile_pool(name="xp", bufs=1))
            big = ctx.enter_context(tc.tile_pool(name="big", bufs=1))
            wqs = ctx.enter_context(tc.tile_pool(name="wqs", bufs=8))
            wvs = ctx.enter_context(tc.tile_pool(name="wvs", bufs=8))
            atp = ctx.enter_context(tc.tile_pool(name="atp", bufs=4))
            etp = ctx.enter_context(tc.tile_pool(name="etp", bufs=8))
            ysb = ctx.enter_context(tc.tile_pool(name="ysb", bufs=4))
            rcp = ctx.enter_context(tc.tile_pool(name="rcp", bufs=4))
            pp = ctx.enter_context(tc.tile_pool(name="pp", bufs=2, space="PSUM"))
            sp = ctx.enter_context(tc.tile_pool(name="sp", bufs=3, space="PSUM"))
            ap_ = ctx.enter_context(tc.tile_pool(name="ap", bufs=2, space="PSUM"))
            rp = ctx.enter_context(tc.tile_pool(name="rp", bufs=1, space="PSUM"))

            # ---- constants ----
            ident = const.tile([128, 128], F16)
            make_identity(nc, ident[:])
            ones64 = const.tile([128, 64], F16)
            nc.vector.memset(ones64[:], 1.0)
            u1t = const.tile([128, L], F16)
            nc.sync.dma_start(u1t[:], U1)
            u0t = const.tile([128, L], F16)
            nc.sync.dma_start(u0t[:], U0)
            bbt = const.tile([128, D], FP)
            nc.sync.dma_start(bbt[:], BB)
            pet = const.tile([HD, L], F16)
            nc.sync.dma_start(pet[:], peT)
            wqpt = const.tile([HD, 2 * D], F16)
            nc.sync.dma_start(wqpt[:], wqp)

            # ---- phase 1: load x, build xT [d, t] (free: dc*512 + tc*128) ----
            xtok = xpool.tile([128, 4 * D], F16)  # [:, tc*1024 : +1024]
            for tc_ in range(4):
                nc.sync.dma_start(
                    xtok[:, tc_ * D : (tc_ + 1) * D], x[tc_ * 128 : (tc_ + 1) * 128, :]
                )
            xT = xpool.tile([128, 8 * 512], F16)
            for dc in range(8):
                pb = pp.tile([128, 512], F16, tag="pp")
                for tc_ in range(4):
                    nc.tensor.transpose(
                        pb[:, tc_ * 128 : (tc_ + 1) * 128],
                        xtok[:, tc_ * D + dc * 128 : tc_ * D + (dc + 1) * 128],
                        ident[:],
                    )
                nc.vector.tensor_copy(xT[:, dc * 512 : (dc + 1) * 512], pb[:])

            # ---- phase 3: v projection, token-major, ones-augmented (bf16) ----
            # vaug: [128, 4*16*65]; (tc, h) block at (tc*16+h)*65, col 64 = 1.0
            vaug = big.tile([128, 4 * 16 * 65], BF)
            v3 = vaug[:].rearrange("p (c e) -> p c e", e=65)
            nc.vector.memset(v3[:, :, 64:65], 1.0)
            for nv in range(2):
                wvt = []
                for kc in range(8):
                    wv = wvs.tile([128, 512], F16, tag="wv")
                    nc.sync.dma_start(
                        wv[:],
                        wqkv[
                            kc * 128 : (kc + 1) * 128,
                            2 * D + nv * 512 : 2 * D + (nv + 1) * 512,
                        ],
                    )
                    wvt.append(wv)
                for tc_ in range(4):
                    vp = pp.tile([128, 512], FP, tag="pp")
                    for kc in range(8):
                        nc.tensor.matmul(
                            vp[:],
                            xT[:, kc * 512 + tc_ * 128 : kc * 512 + tc_ * 128 + 128],
                            wvt[kc][:],
                            start=(kc == 0),
                            stop=(kc == 7),
                        )
                    nc.vector.tensor_copy(
                        v3[:, tc_ * 16 + nv * 8 : tc_ * 16 + (nv + 1) * 8, 0:64], vp[:]
                    )

            # ---- A'^T tiles (gauss + key padding - row stabilizer), loaded once ----
            at_sb = []
            for jt in range(4):
                a = atp.tile([128, 512], F16)
                nc.sync.dma_start(a[:], at[jt * 128 : (jt + 1) * 128, :])
                at_sb.append(a)

            oaT = big.tile([128, 8 * 512], F16)  # out_attn^T, feature-major

            # ---- phases 2,4,5-7 per head-group of 8 ----
            # Within a group: local head hl at rows (hl%2)*64..+64,
            # cols (hl//2)*512..+512 of Q1/Q0/K1/K0/QP1/QP0/KP1/KP0.
            for g in range(2):
                Q1 = big.tile([128, 4 * 512], F16, tag="Q1")
                Q0 = big.tile([128, 4 * 512], F16, tag="Q0")
                K1 = big.tile([128, 4 * 512], F16, tag="K1")
                K0 = big.tile([128, 4 * 512], F16, tag="K0")
                QP1 = big.tile([128, 4 * 512], F16, tag="QP1")
                QP0 = big.tile([128, 4 * 512], F16, tag="QP0")
                KP1 = big.tile([128, 4 * 512], F16, tag="KP1")
                KP0 = big.tile([128, 4 * 512], F16, tag="KP0")

                # q/k projection (feature-major) + speaker-mask fused copies
                for fpt in range(8):
                    # local feature-tiles 0-3 = q of heads g*8.., 4-7 = k
                    isq = fpt < 4
                    p = fpt if isq else fpt - 4
                    col = (0 if isq else D) + g * 512 + p * 128
                    qp_ps = pp.tile([128, 512], FP, tag="pp")
                    for kc in range(8):
                        wt = wqs.tile([128, 128], F16)
                        nc.sync.dma_start(
                            wt[:], wqkv[kc * 128 : (kc + 1) * 128, col : col + 128]
                        )
                        nc.tensor.matmul(
                            qp_ps[:],
                            wt[:],
                            xT[:, kc * 512 : (kc + 1) * 512],
                            start=(kc == 0),
                            stop=(kc == 7),
                        )
                    d1, d0 = (Q1, Q0) if isq else (K1, K0)
                    sl = slice(p * 512, (p + 1) * 512)
                    nc.vector.tensor_mul(d1[:, sl], qp_ps[:], u1t[:])
                    nc.vector.tensor_mul(d0[:, sl], qp_ps[:], u0t[:])

                # positional projection (feature-major) + mask fused copies
                for fpt in range(8):
                    isq = fpt < 4
                    p = fpt if isq else fpt - 4
                    col = (0 if isq else D) + g * 512 + p * 128
                    yp = pp.tile([128, 512], FP, tag="pp")
                    nc.tensor.matmul(
                        yp[:],
                        wqpt[:, col : col + 128],
                        pet[:],
                        start=True,
                        stop=True,
                    )
                    d1, d0 = (QP1, QP0) if isq else (KP1, KP0)
                    sl = slice(p * 512, (p + 1) * 512)
                    nc.vector.tensor_mul(d1[:, sl], yp[:], u1t[:])
                    nc.vector.tensor_mul(d0[:, sl], yp[:], u0t[:])

                # scores^T -> exp -> AV -> normalize, per local head
                for hl in range(8):
                    h = g * 8 + hl
                    hb = (hl % 2) * 64
                    hp = (hl // 2) * 512
                    ets = []
                    for jt in range(4):
                        s_ps = sp.tile([128, 512], FP, tag="sp")
                        jsl = slice(hp + jt * 128, hp + jt * 128 + 128)
                        isl = slice(hp, hp + 512)
                        nc.tensor.matmul(
                            s_ps[:], K1[hb : hb + 64, jsl], Q1[hb : hb + 64, isl],
                            start=True, stop=False,
                        )
                        nc.tensor.matmul(
                            s_ps[:], K0[hb : hb + 64, jsl], Q0[hb : hb + 64, isl],
                            start=False, stop=False,
                        )
                        nc.tensor.matmul(
                            s_ps[:], KP1[hb : hb + 64, jsl], QP1[hb : hb + 64, isl],
                            start=False, stop=False,
                        )
                        nc.tensor.matmul(
                            s_ps[:], KP0[hb : hb + 64, jsl], QP0[hb : hb + 64, isl],
                            start=False, stop=False,
                        )
                        nc.tensor.matmul(
                            s_ps[:], ident[:], at_sb[jt][:],
                            start=False, stop=True,
                        )
                        e_t = etp.tile([128, 512], BF)
                        nc.scalar.activation(e_t[:], s_ps[:], AF.Exp)
                        ets.append(e_t)

                    av = ap_.tile([128, 512], FP, tag="ap")
                    for jt in range(4):
                        base = jt * 16 * 65 + h * 65
                        nc.tensor.matmul(
                            av[0:65, :],
                            vaug[:, base : base + 65],
                            ets[jt][:],
                            start=(jt == 0),
                            stop=(jt == 3),
                        )
                    rec = rcp.tile([128, 512], F16)
                    nc.vector.reciprocal(rec[64:65, :], av[64:65, :])
                    rb = rp.tile([64, 512], FP, tag="rp")
                    nc.tensor.matmul(
                        rb[:], ones64[64:65, 0:64], rec[64:65, :],
                        start=True, stop=True,
                    )
                    rbs = rcp.tile([64, 512], FP, tag="rbs")
                    nc.scalar.copy(rbs[:], rb[:])
                    ob = (h % 2) * 64
                    op = (h // 2) * 512
                    nc.vector.tensor_mul(
                        oaT[ob : ob + 64, op : op + 512], av[0:64, :], rbs[:]
                    )

            # ---- phase 8: FC + bias ----
            for ne in range(2):
                wft = []
                for fc8 in range(8):
                    wf = wvs.tile([128, 512], F16, tag="wv")
                    nc.sync.dma_start(
                        wf[:],
                        wfc[fc8 * 128 : (fc8 + 1) * 128, ne * 512 : (ne + 1) * 512],
                    )
                    wft.append(wf)
                for tc_ in range(4):
                    yp_ = ap_.tile([128, 512], FP, tag="ap")
                    for fc8 in range(8):
                        nc.tensor.matmul(
                            yp_[:],
                            oaT[:, fc8 * 512 + tc_ * 128 : fc8 * 512 + tc_ * 128 + 128],
                            wft[fc8][:],
                            start=(fc8 == 0),
                            stop=(fc8 == 7),
                        )
                    y_t = ysb.tile([128, 512], FP)
                    nc.vector.scalar_tensor_tensor(
                        y_t[:], yp_[:], 1.0, bbt[:, ne * 512 : (ne + 1) * 512],
                        op0=OP.mult, op1=OP.add,
                    )
                    nc.sync.dma_start(
                        y[tc_ * 128 : (tc_ + 1) * 128, ne * 512 : (ne + 1) * 512],
                        y_t[:],
                    )
    return nc


def host_prep(x, mask, qmask, w_qkv, w_qkpos, w_fc, b_fc, shift, bias):
    """Build per-core input maps (host-side numpy only)."""
    x = np.asarray(x, np.float32)
    mask = np.asarray(mask)
    qmask = np.asarray(qmask)
    b_fc = np.asarray(b_fc, np.float32)
    shift = float(np.asarray(shift).reshape(-1)[0])
    bias = float(np.asarray(bias).reshape(-1)[0])
    wqkv16 = np.asarray(w_qkv).astype(np.float16)
    wqp16 = np.asarray(w_qkpos).astype(np.float16)
    wfc16 = np.asarray(w_fc).astype(np.float16)

    half = HD // 2
    inv = np.exp(np.arange(half, dtype=np.float64) * (-(np.log(10000.0) / (half - 1))))
    r = np.arange(-(L // 2), L // 2, dtype=np.float64)
    ang = r[:, None] * inv[None, :]
    pe = np.concatenate([np.sin(ang), np.cos(ang)], axis=1).astype(np.float32)
    peT16 = np.ascontiguousarray(pe.T).astype(np.float16)  # (HD, L)

    idx = np.arange(L, dtype=np.float32)
    sqd = (idx[:, None] - idx[None, :]) ** 2
    G = -(shift * sqd + bias)  # (L, L), symmetric

    BBrow = np.ascontiguousarray(
        np.broadcast_to(b_fc[None, :], (128, D)).astype(np.float32)
    )

    in_maps = []
    for b in range(B):
        kneg = np.where(mask[b] == 0, np.float32(-1.0e9), np.float32(0.0))
        c_base = (G + kneg[None, :]).max(axis=1)  # max over valid j
        aT = (G + kneg[:, None] - c_base[None, :]).astype(np.float32)  # [j, i]
        # clamp to finite fp16 (avoid 0*inf=NaN in the identity matmul);
        # -60000 + |S|max is still << ln(fp32 min), so exp -> 0 exactly
        aT16 = np.clip(aT, -60000.0, None).astype(np.float16)
        u = qmask[b].astype(np.float16)
        U1 = np.ascontiguousarray(np.broadcast_to(u[None, :], (128, L)))
        U0 = np.ascontiguousarray(np.broadcast_to((1 - u)[None, :], (128, L)))
        in_maps.append(
            dict(
                x=np.ascontiguousarray(x[b]).astype(np.float16),
                wqkv=wqkv16,
                wqp=wqp16,
                wfc=wfc16,
                peT=peT16,
                at=np.ascontiguousarray(aT16),
                U1=U1,
                U0=U0,
                BB=BBrow,
            )
        )
    return in_maps


_NC_CACHE = {}


def get_nc():
    if "nc" not in _NC_CACHE:
        nc = bacc.Bacc(
            "TRN2", target_bir_lowering=False, debug=False, enable_asserts=False,
            num_devices=B,
        )
        build_kernel(nc)
        nc.compile()
        _NC_CACHE["nc"] = nc
    return _NC_CACHE["nc"]


def kernel(**inputs):
    from concourse import bass_utils

    in_maps = host_prep(**inputs)
    nc = get_nc()
    res = bass_utils.run_bass_kernel_spmd(nc, in_maps, list(range(B)))
    out = np.stack([m["y"] for m in res.results], axis=0)
    return out.astype(np.float32)


if __name__ == "__main__":
    rng = np.random.default_rng(0)
    ins = dict(
        x=rng.standard_normal((B, L, D), dtype=np.float32),
        mask=rng.integers(0, 2, (B, L)).astype(np.int64),
        qmask=rng.integers(0, 2, (B, L)).astype(np.int64),
        w_qkv=(rng.standard_normal((D, 3 * D), dtype=np.float32) * 0.02),
        w_qkpos=(rng.standard_normal((HD, 2 * D), dtype=np.float32) * 0.02),
        w_fc=(rng.standard_normal((D, D), dtype=np.float32) * 0.02),
        b_fc=np.zeros((D,), np.float32),
        shift=np.abs(rng.standard_normal(1)).astype(np.float32) + 0.001,
        bias=-np.abs(rng.standard_normal(1)).astype(np.float32),
    )
    ins["mask"][:, 0] = 1
    out = kernel(**ins)
    print(out.shape, out.dtype)
